# revision 76
# baseline (speedup 1.0000x reference)
"""Trainium2 Bass kernel for CausalSelfAttention (GQA + QK-RMSNorm + RoPE).

Problem shapes (hardcoded): B=2, S=2048, D=2048, H=16, KVH=4, HD=128.

Sharding: 8 cores = 2 batches x 4 kv-head groups. Core c handles batch
b = c // 4 and kv-group g = c % 4 (q-heads 4g..4g+3, kv head g). Each core
computes its 4 heads end-to-end plus a partial output projection over its
512 columns of Wproj's input dim; the host sums the 4 partials per batch.

fp8 DoubleRow strategy (PE matmuls at 0.5 cycles/row in DR mode):
  * Host sends x as fp8 (hi, lo) byte pairs and all weights as fp8 hi+lo
    planes pre-scaled by 64 (fp8 e4m3 loses precision below ~2^-6; the
    64x cancels inside RMS-norm on the q/k paths and is divided out on
    the host for the v/proj path).
  * QKV projections: "3-term" hi/lo products
        x @ W ~= (x_hi + x_lo) @ W_hi  +  x_hi @ W_lo
    term12 is one DoubleRow matmul per d-tile (stationary slabs =
    (x_hi, x_lo), moving = W_hi slab-broadcast), term3 one DoubleRow
    matmul per d-tile PAIR.  0.625x the bf16 row count, ~bf16 accuracy.
  * Scores: half-DR: stationary slabs = (k_hi, k_lo) (k exact), moving =
    fp8 q (x16 pre-scale, undone via exp scale=1/16).  0.5x rows; the
    one meaningful fp8 error site (~1e-2).
  * PV + softmax denominator: bf16 (fp8 on exp(scores)/V costs >2e-2).
  * Output projection: 3-term with y split hi/lo on-device (scaled 64x
    via the V path) and Wproj hi/lo from host: 0.75x rows.

Softmax rsqrt runs as exp(-0.5*ln(.)) so Ln/Exp share one ACT function
table (no table reloads in the attention exp stream).  Attention is
processed in HEAD PAIRS: one exp instruction covers both heads of a
pair via a [P, 2, 512] score-pair PSUM tile.  Attention for q-block qb
drips into phase 1 once its s-tiles are ~2 tiles old (ATT_SLACK), and
projection units follow each q-block's attention.  All DoubleRow hi/lo
operands use plane-separated layouts (slab strides even + 16B aligned,
an ISA requirement).  PSUM banks (8): psA 2x[P,2,512] (phase-1 QKV
psums / score pairs), od 2 (attn-out), psD 2 (denominator + out-proj
accumulators, time-shared via one pool tag).
"""

import numpy as np

B, S, D = 2, 2048, 2048
H, KVH = 16, 4
HD = D // H            # 128
NH = H // KVH          # 4 heads per core
P = 128
ST = S // P            # 16 s-tiles
DT = D // P            # 16 d-tiles
FT = NH * HD // P      # 4 f-tiles (proj contraction per core)
QB = 512               # q-block width in phase 2
NQB = S // QB          # 4
SBW = 256              # phase-1 x DMA block width (s columns)
GRP = 8                # k-tiles per softmax-denominator group
ROPE_BASE = 10000.0
EPS = 1e-6
WSCALE = 64.0          # fp8 pre-scale on all weights
QSCALE = 16.0          # extra pre-scale on q before fp8 (undone in exp)
OUT_DESCALE = 1.0 / (WSCALE * WSCALE)   # host-side, on gathered output

# drip-feed budgets per s-tile (index 0..15): attention k-tile units and
# projection column-tile units interleaved into phase 1.  ATT_SLACK keeps
# dripped units' inputs a couple of s-tiles ahead so their waits are
# satisfied at issue (in-order engine queues head-of-line block otherwise).
ATT_SLACK = 2
ATT_BUDGET = [0, 0, 0, 0, 0, 2, 2, 2, 2, 2, 2, 2, 2, 3, 3, 3]
PROJ_BUDGET = [0, 0, 0, 0, 0, 0, 0, 0, 0, 0, 2, 2, 2, 2, 2, 2]

_CACHE = {}


def _build_nc():
    from collections import deque
    from contextlib import ExitStack

    import concourse.mybir as mybir
    import concourse.tile as tile
    from concourse import bacc

    f32 = mybir.dt.float32
    bf16 = mybir.dt.bfloat16
    f8 = mybir.dt.float8e4
    AF = mybir.ActivationFunctionType
    MUL = mybir.AluOpType.mult
    ADD = mybir.AluOpType.add
    SUB = mybir.AluOpType.subtract
    DR = mybir.MatmulPerfMode.DoubleRow
    AXX = mybir.AxisListType.X

    nc = bacc.Bacc("TRN2", target_bir_lowering=False, debug=False, num_devices=8)

    xp = nc.dram_tensor("xp", [D, 2, S], f8, kind="ExternalInput").ap()
    wq8 = nc.dram_tensor("wq8", [D, NH * HD], f8, kind="ExternalInput").ap()
    wql = nc.dram_tensor("wql", [D, NH * HD], f8, kind="ExternalInput").ap()
    wkv8 = nc.dram_tensor("wkv8", [D, 2 * HD], f8, kind="ExternalInput").ap()
    wkvl = nc.dram_tensor("wkvl", [D, 2 * HD], f8, kind="ExternalInput").ap()
    wph = nc.dram_tensor("wph", [NH * HD, D], f8, kind="ExternalInput").ap()
    wpl = nc.dram_tensor("wpl", [NH * HD, D], f8, kind="ExternalInput").ap()
    cs2 = nc.dram_tensor("cs2", [S, 2 * HD], bf16, kind="ExternalInput").ap()
    qg5 = nc.dram_tensor("qg5", [P, NH + 1], f32, kind="ExternalInput").ap()
    tri = nc.dram_tensor("tri", [P, P], bf16, kind="ExternalInput").ap()
    onesd = nc.dram_tensor("onesd", [P, P], bf16, kind="ExternalInput").ap()
    outT = nc.dram_tensor("outT", [D, S], bf16, kind="ExternalOutput").ap()

    with tile.TileContext(nc) as tc:
        with ExitStack() as octx:
            const = octx.enter_context(tc.tile_pool(name="const", bufs=1))
            big = octx.enter_context(tc.tile_pool(name="big", bufs=1))
            wpool = octx.enter_context(tc.tile_pool(name="wpool", bufs=1))
            xpool = octx.enter_context(tc.tile_pool(name="xpool", bufs=2))
            stq = octx.enter_context(tc.tile_pool(name="stq", bufs=3))
            stk = octx.enter_context(tc.tile_pool(name="stk", bufs=3))
            sml = octx.enter_context(tc.tile_pool(name="sml", bufs=3))
            expool = octx.enter_context(tc.tile_pool(name="expool", bufs=8))
            dent = octx.enter_context(tc.tile_pool(name="dent", bufs=3))
            recpool = octx.enter_context(tc.tile_pool(name="recpool", bufs=2))
            ytpool = octx.enter_context(tc.tile_pool(name="ytpool", bufs=2))
            ostage = octx.enter_context(tc.tile_pool(name="ostage", bufs=4))
            psA = octx.enter_context(
                tc.tile_pool(name="psA", bufs=2, space="PSUM"))
            od = octx.enter_context(
                tc.tile_pool(name="od", bufs=2, space="PSUM"))
            psD = octx.enter_context(
                tc.tile_pool(name="psD", bufs=2, space="PSUM"))

            # ---- persistent stores ----
            QT = big.tile([P, NH, S], bf16)     # q^T rope'd (x16 gain): [hd, h, s]
            QT8 = big.tile([P, NH, S], f8)      # fp8 copy of QT
            KT = big.tile([P, S], bf16)         # k^T: [hd, s]
            KTp = big.tile([P, 2, S], f8)       # k^T hi/lo planes
            VS = big.tile([P, ST, HD], bf16)    # v (x64): [s-part, s-tile, hd]
            YTp = big.tile([P, NH, 2, S], f8)   # attn out^T (x64) hi/lo planes

            xpr = xp.rearrange("(dt p) two s -> p dt two s", p=P)
            wq8r = wq8.rearrange("(dt p) e -> p dt e", p=P)
            wqlr = wql.rearrange("(dt p) e -> p dt e", p=P)
            wkv8r = wkv8.rearrange("(dt p) e -> p dt e", p=P)
            wkvlr = wkvl.rearrange("(dt p) e -> p dt e", p=P)
            outTr = outT.rearrange("(et p) s -> p et s", p=P)

            WQ8 = wpool.tile([P, DT, NH * HD], f8)
            WQL = wpool.tile([P, DT, NH * HD], f8)
            WKV8 = wpool.tile([P, DT, 2 * HD], f8)
            WKVL = wpool.tile([P, DT, 2 * HD], f8)
            WPH = wpool.tile([P, FT, D], f8)
            WPL = wpool.tile([P, FT, D], f8)

            def load_xblk(sb, nchunk=2):
                t = xpool.tile([P, DT, 2, SBW], f8, tag="xblk", name="xblk")
                step = DT // nchunk
                ssl_ = slice(sb * SBW, (sb + 1) * SBW)
                for c in range(0, DT, step):
                    for pl in range(2):
                        nc.sync.dma_start(
                            t[:, c:c + step, pl],
                            xpr[:, c:c + step, pl, ssl_])
                return t

            # startup: interleave small WQ / x0 pieces so the first Q
            # matmuls can start after ~2 chunks; the rest follows.
            xblk_next = xpool.tile([P, DT, 2, SBW], f8, tag="xblk", name="xblk")
            for c0, c1 in ((0, 2), (2, 4), (4, 8), (8, 12), (12, 16)):
                nc.sync.dma_start(WQ8[:, c0:c1, :], wq8r[:, c0:c1, :])
                nc.sync.dma_start(
                    xblk_next[:, c0:c1, 0], xpr[:, c0:c1, 0, 0:SBW])
                nc.sync.dma_start(
                    xblk_next[:, c0:c1, 1], xpr[:, c0:c1, 1, 0:SBW])
            nc.sync.dma_start(WKV8[:], wkv8r[:])
            nc.sync.dma_start(WQL[:], wqlr[:])
            nc.sync.dma_start(WKVL[:], wkvlr[:])

            # ---- constants ----
            cs_t = const.tile([P, ST, 2 * HD], bf16)
            nc.sync.dma_start(cs_t[:], cs2.rearrange("(st p) c -> p st c", p=P))
            qg_t = const.tile([P, NH + 1], f32)
            nc.sync.dma_start(qg_t[:], qg5)
            tri_t = const.tile([P, P], bf16)
            nc.sync.dma_start(tri_t[:], tri)
            ones_t = const.tile([P, P], bf16)
            nc.sync.dma_start(ones_t[:], onesd)
            eps_t = const.tile([P, 1], f32)
            nc.vector.memset(eps_t[:], HD * WSCALE * WSCALE * EPS)

            # ------ attention emitter (one yield per k-tile, HEAD PAIRS) ----
            # One DoubleRow scores matmul + one exp covers both heads of a
            # pair (out [P, 2, 512] spans two PSUM banks), halving the
            # attention stream's instruction count and cross-engine edges.
            def att_units(qb):
                nk = NH * qb + NH
                for hp in range(NH // 2):
                    h0 = 2 * hp
                    qsl = slice(qb * QB, (qb + 1) * QB)
                    oT0 = od.tile([P, QB], f32, tag="o", name="oT0")
                    oT1 = od.tile([P, QB], f32, tag="o", name="oT1")
                    den0 = psD.tile([P, QB], f32, tag="d", name="den0")
                    den1 = psD.tile([P, QB], f32, tag="d", name="den1")
                    ex_prev, q0_prev = None, 0
                    for kt in range(nk):
                        j = kt - NH * qb  # >= 0 on diagonal tiles
                        q0 = P * j if j >= 0 else 0
                        w = QB - q0
                        ps2 = psA.tile([P, 2, QB], f32, tag="ps", name="ps2")
                        for hh in range(2):
                            nc.tensor.matmul(
                                ps2[:, hh, q0:QB],
                                KTp[:, :, kt * P:(kt + 1) * P],
                                QT8[:, h0 + hh, qb * QB + q0:(qb + 1) * QB][
                                    :, None, :].to_broadcast([P, 2, w]),
                                start=True, stop=True, perf_mode=DR)
                        ex2 = expool.tile([P, 2, QB], bf16, tag="ex",
                                          name="ex2")
                        if j >= 1:
                            nc.gpsimd.memset(
                                ex2[:, :, 0:q0].bitcast(mybir.dt.uint16), 0)
                        nc.scalar.activation(
                            ex2[:, :, q0:QB], ps2[:, :, q0:QB], AF.Exp,
                            scale=1.0 / QSCALE)
                        if j >= 0:
                            nc.vector.tensor_tensor(
                                ex2[:, :, q0:q0 + P], ex2[:, :, q0:q0 + P],
                                tri_t[:, None, :].to_broadcast([P, 2, P]),
                                MUL)
                        nc.tensor.matmul(
                            oT0[:, q0:QB], VS[:, kt], ex2[:, 0, q0:QB],
                            start=(kt == 0), stop=(kt == nk - 1))
                        nc.tensor.matmul(
                            oT1[:, q0:QB], VS[:, kt], ex2[:, 1, q0:QB],
                            start=(kt == 0), stop=(kt == nk - 1))
                        # denominator: one DVE pair-sum (both heads) + two
                        # ones-matmuls per two k-tiles.
                        if kt % 2 == 0:
                            ex_prev, q0_prev = ex2, q0
                        else:
                            psum2 = dent.tile([P, 2, QB], bf16, tag="dt",
                                              name="psum2")
                            nc.vector.tensor_tensor(
                                psum2[:, :, q0_prev:QB],
                                ex_prev[:, :, q0_prev:QB],
                                ex2[:, :, q0_prev:QB], ADD)
                            nc.tensor.matmul(
                                den0[:, q0_prev:QB], ones_t[:],
                                psum2[:, 0, q0_prev:QB],
                                start=(kt == 1), stop=(kt == nk - 1))
                            nc.tensor.matmul(
                                den1[:, q0_prev:QB], ones_t[:],
                                psum2[:, 1, q0_prev:QB],
                                start=(kt == 1), stop=(kt == nk - 1))
                        yield
                    for h, oT, den in ((h0, oT0, den0), (h0 + 1, oT1, den1)):
                        rec = recpool.tile([P, QB], f32, tag="rec",
                                           name="rec")
                        nc.vector.reciprocal_approx_fast(rec[:], den[:])
                        yt = ytpool.tile([P, QB], bf16, tag="yt", name="yt")
                        nc.vector.tensor_tensor(yt[:], oT[:], rec[:], MUL)
                        nc.vector.tensor_copy(YTp[:, h, 0, qsl], yt[:])
                        nc.vector.tensor_tensor(
                            YTp[:, h, 1, qsl], yt[:], YTp[:, h, 0, qsl], SUB)

            # ---------- proj unit emitter (one output column-tile) ----------
            obcur = [None]

            def proj_unit(sb3, et):
                # in the tail (last q-block) attention is done, so both the
                # od and psD pools are free: alternate accumulators across
                # them to double the drain parallelism.
                if sb3 == NQB - 1 and et % 3 == 0:
                    po = od.tile([P, QB], f32, tag="o", name="po")
                elif sb3 == NQB - 1 and et % 3 == 1:
                    pot = psA.tile([P, 2, QB], f32, tag="ps", name="pot")
                    po = pot[:, 0]
                else:
                    po = psD.tile([P, QB], f32, tag="d", name="po")
                esl = slice(et * P, (et + 1) * P)
                qsl = slice(sb3 * QB, (sb3 + 1) * QB)
                for ft in range(FT):
                    # term12: Wp_hi^T (y_hi + y_lo)
                    nc.tensor.matmul(
                        po[:],
                        WPH[:, ft, esl][:, None, :].to_broadcast([P, 2, P]),
                        YTp[:, ft, :, qsl],
                        start=(ft == 0), stop=False, perf_mode=DR,
                        skip_group_check=True)
                for t2 in range(FT // 2):
                    # term3: Wp_lo^T y_hi, two f-tiles per matmul
                    nc.tensor.matmul(
                        po[:],
                        WPL[:, 2 * t2:2 * t2 + 2, esl],
                        YTp[:, 2 * t2:2 * t2 + 2, 0, qsl],
                        start=False, stop=(t2 == FT // 2 - 1), perf_mode=DR,
                        skip_group_check=True)
                if et % 2 == 0:
                    obcur[0] = ostage.tile([P, 2, QB], bf16, tag="ob",
                                           name="ob")
                    nc.scalar.copy(obcur[0][:, 0], po[:])
                else:
                    nc.vector.tensor_copy(obcur[0][:, 1], po[:])
                    nc.sync.dma_start(
                        outTr[:, et - 1:et + 1,
                              sb3 * QB:(sb3 + 1) * QB], obcur[0][:])

            # scheduling state: attention generators + proj queues
            att_gens = [att_units(qb) for qb in range(NQB)]
            att_left = [(NH // 2) * (NH * qb + NH) for qb in range(NQB)]
            cur_att = [0]          # current qb being drained
            proj_pending = deque()
            proj_unlocked = [0]    # qb < this: attention fully consumed

            def note_attention_progress():
                while cur_att[0] < NQB and att_left[cur_att[0]] == 0:
                    qb_done = cur_att[0]
                    proj_pending.extend((qb_done, et) for et in range(DT))
                    cur_att[0] += 1

            def run_att(n, st_limit):
                done = 0
                while done < n and cur_att[0] < NQB:
                    qb = cur_att[0]
                    if 4 * qb + 3 + ATT_SLACK > st_limit:
                        break
                    next(att_gens[qb])
                    att_left[qb] -= 1
                    done += 1
                    if att_left[qb] == 0:
                        # flush the final head's normalize epilogue
                        try:
                            next(att_gens[qb])
                        except StopIteration:
                            pass
                    note_attention_progress()
                return done

            def run_proj(n):
                done = 0
                while done < n and proj_pending:
                    sb3, et = proj_pending.popleft()
                    proj_unit(sb3, et)
                    done += 1
                return done

            # =========================== Phase 1 ===========================
            for sb in range(S // SBW):
                xblk = xblk_next
                if sb + 1 < S // SBW:
                    xblk_next = load_xblk(sb + 1)
                if sb == 2:
                    nc.sync.dma_start(
                        WPH[:], wph.rearrange("(ft p) e -> p ft e", p=P))
                    nc.sync.dma_start(
                        WPL[:], wpl.rearrange("(ft p) e -> p ft e", p=P))
                for jj in range(SBW // P):
                    st = sb * (SBW // P) + jj
                    ssl = slice(jj * P, (jj + 1) * P)

                    if jj % 2 == 0:
                        qk2 = psA.tile([P, 2, QB], f32, tag="ps", name="qk2")
                        kv2 = psA.tile([P, 2, QB], f32, tag="ps", name="kv2")
                    psq = qk2[:, jj % 2]
                    pskv = kv2[:, jj % 2, 0:2 * HD]
                    for dt in range(DT):
                        nc.tensor.matmul(
                            psq[:],
                            xblk[:, dt, :, ssl],
                            WQ8[:, dt][:, None, :].to_broadcast(
                                [P, 2, NH * HD]),
                            start=(dt == 0), stop=False, perf_mode=DR,
                            skip_group_check=True)
                    for t2 in range(DT // 2):
                        nc.tensor.matmul(
                            psq[:],
                            xblk[:, 2 * t2:2 * t2 + 2, 0, ssl],
                            WQL[:, 2 * t2:2 * t2 + 2, :],
                            start=False, stop=(t2 == DT // 2 - 1),
                            perf_mode=DR, skip_group_check=True)

                    for dt in range(DT):
                        nc.tensor.matmul(
                            pskv[:, 0:2 * HD],
                            xblk[:, dt, :, ssl],
                            WKV8[:, dt][:, None, :].to_broadcast(
                                [P, 2, 2 * HD]),
                            start=(dt == 0), stop=False, perf_mode=DR,
                            skip_group_check=True)
                    for t2 in range(DT // 2):
                        nc.tensor.matmul(
                            pskv[:, 0:2 * HD],
                            xblk[:, 2 * t2:2 * t2 + 2, 0, ssl],
                            WKVL[:, 2 * t2:2 * t2 + 2, :],
                            start=False, stop=(t2 == DT // 2 - 1),
                            perf_mode=DR, skip_group_check=True)

                    # stage psums to bf16 SBUF immediately (frees the PSUM
                    # slots so phase-1 pipelining is not chained to the
                    # rms-norm latency)
                    qf = stq.tile([P, NH, HD], bf16, tag="qf")
                    nc.scalar.copy(
                        qf[:], psq[:].rearrange("p (h c) -> p h c", h=NH))
                    kf = stk.tile([P, HD], bf16, tag="kf")
                    nc.scalar.copy(kf[:], pskv[:, 0:HD])
                    # V (x64) straight to its bf16 store
                    nc.vector.tensor_copy(VS[:, st], pskv[:, HD:2 * HD])

                    # -- fused Q+K rmsnorm statistics --
                    sqscr = stq.tile([P, NH, HD], bf16, tag="qsq")
                    nc.gpsimd.tensor_tensor(sqscr[:], qf[:], qf[:], MUL)
                    skscr = stk.tile([P, HD], bf16, tag="ksq")
                    nc.gpsimd.tensor_tensor(skscr[:], kf[:], kf[:], MUL)
                    ssqk = sml.tile([P, NH + 1], f32, tag="ssq")
                    nc.vector.tensor_reduce(ssqk[:, 0:NH], sqscr[:], AXX, ADD)
                    nc.vector.tensor_reduce(
                        ssqk[:, NH:NH + 1], skscr[:, None, :], AXX, ADD)
                    # rsq = (ssqk + HD*eps')^(-1/2) via exp(-0.5*ln(.)):
                    # Ln and Exp share one ACT function table, so the
                    # attention exp stream never reloads tables.  The
                    # 1/sqrt(HD) and x64 weight scales fold into qg (host).
                    lnt = sml.tile([P, NH + 1], f32, tag="lnt")
                    nc.scalar.activation(lnt[:], ssqk[:], AF.Ln,
                                         bias=eps_t[:])
                    rsq = sml.tile([P, NH + 1], f32, tag="rsq")
                    nc.scalar.activation(rsq[:], lnt[:], AF.Exp, scale=-0.5)
                    rsg = sml.tile([P, NH + 1], f32, tag="rsg")
                    nc.vector.tensor_tensor(rsg[:], rsq[:], qg_t[:], MUL)

                    # -- Q rope (gain x16 folded into rsg) --
                    qn = stq.tile([P, NH, HD], bf16, tag="qn")
                    for hh in range(NH):
                        nc.vector.tensor_scalar(
                            qn[:, hh], qf[:, hh], rsg[:, hh:hh + 1], None,
                            MUL)
                    cos_bc = cs_t[:, st:st + 1, 0:HD].to_broadcast([P, NH, HD])
                    qa = stq.tile([P, NH, HD], bf16, tag="qa")
                    nc.vector.tensor_tensor(qa[:], qn[:], cos_bc, MUL)
                    qb_ = stq.tile([P, NH, HD], bf16, tag="qb")
                    nc.vector.tensor_tensor(
                        qb_[:, :, 0:HD // 2], qn[:, :, HD // 2:HD],
                        cs_t[:, st:st + 1, HD:HD + HD // 2].to_broadcast(
                            [P, NH, HD // 2]), MUL)
                    nc.vector.tensor_tensor(
                        qb_[:, :, HD // 2:HD], qn[:, :, 0:HD // 2],
                        cs_t[:, st:st + 1, HD + HD // 2:2 * HD].to_broadcast(
                            [P, NH, HD // 2]), MUL)
                    qrot = stq.tile([P, NH, HD], bf16, tag="qr")
                    nc.vector.tensor_tensor(qrot[:], qa[:], qb_[:], ADD)

                    # -- K rmsnorm + rope --
                    kn = stk.tile([P, HD], bf16, tag="kn")
                    nc.vector.tensor_scalar(
                        kn[:], kf[:], rsg[:, NH:NH + 1], None, MUL)
                    ka = stk.tile([P, HD], bf16, tag="ka")
                    nc.vector.tensor_tensor(ka[:], kn[:], cs_t[:, st, 0:HD], MUL)
                    kb = stk.tile([P, HD], bf16, tag="kb")
                    nc.vector.tensor_tensor(
                        kb[:, 0:HD // 2], kn[:, HD // 2:HD],
                        cs_t[:, st, HD:HD + HD // 2], MUL)
                    nc.vector.tensor_tensor(
                        kb[:, HD // 2:HD], kn[:, 0:HD // 2],
                        cs_t[:, st, HD + HD // 2:2 * HD], MUL)
                    krot = stk.tile([P, HD], bf16, tag="kr")
                    nc.vector.tensor_tensor(krot[:], ka[:], kb[:], ADD)

                    # -- DMA-XBAR transposes into QT / KT --
                    nc.scalar.dma_start_transpose(
                        QT[:, :, st * P:(st + 1) * P], qrot[:])
                    nc.scalar.dma_start_transpose(
                        KT[:, st * P:(st + 1) * P], krot[:])
                    # fp8 conversions on the transposed layouts
                    qsl8 = slice(st * P, (st + 1) * P)
                    from contextlib import nullcontext
                    with (tc.high_priority() if st < 4 else nullcontext()):
                        if st < 5:
                            nc.scalar.copy(QT8[:, :, qsl8], QT[:, :, qsl8])
                        else:
                            nc.gpsimd.tensor_copy(QT8[:, :, qsl8],
                                                  QT[:, :, qsl8])
                        nc.vector.tensor_copy(KTp[:, 0, qsl8], KT[:, qsl8])
                        nc.gpsimd.tensor_tensor(
                            KTp[:, 1, qsl8], KT[:, qsl8], KTp[:, 0, qsl8],
                            SUB)

                    # drip-feed attention + proj units
                    run_att(ATT_BUDGET[st], st)
                    run_proj(PROJ_BUDGET[st])

            # ==================== Phase 2/3 tail ===========================
            # remaining attention with proj interleaved at the global ratio
            total_att = sum(att_left)
            done_att = 0
            emitted = 0
            while cur_att[0] < NQB:
                if run_att(1, ST + ATT_SLACK + 4) == 0:
                    break
                done_att += 1
                # keep proj flowing proportionally (48 proj units interleave
                # into the remaining attention stream; proj3 drains after)
                want = (done_att * 3 * DT) // max(total_att, 1)
                while proj_pending and emitted < want:
                    run_proj(1)
                    emitted += 1
            while proj_pending:
                run_proj(1)

    nc.compile()
    return nc


def _host_inputs(x, Wq, Wk, Wv, Wproj, q_gain):
    """Build the 8 per-core input maps (fp8 hi/lo + bf16 consts)."""
    import ml_dtypes
    bf16 = ml_dtypes.bfloat16
    F8 = ml_dtypes.float8_e4m3
    f32 = np.float32

    def hilo_planes2(a):
        """[D, 2, S] fp8 (hi, lo) planes along a new middle axis."""
        hi = a.astype(F8)
        lo = (a - hi.astype(f32)).astype(F8)
        return np.ascontiguousarray(np.stack([hi, lo], axis=1))

    def hi_lo_planes(a):
        hi = a.astype(F8)
        lo = (a - hi.astype(f32)).astype(F8)
        return (np.ascontiguousarray(hi), np.ascontiguousarray(lo))

    inv_freq = 1.0 / (ROPE_BASE ** (np.arange(0, HD, 2, dtype=f32) / HD))
    freqs = np.outer(np.arange(S, dtype=f32), inv_freq).astype(f32)
    c = np.cos(freqs).astype(f32)
    s = np.sin(freqs).astype(f32)
    cos2 = np.concatenate([c, c], axis=1)
    sin2 = np.concatenate([s, -s], axis=1)
    cs2 = np.ascontiguousarray(
        np.concatenate([cos2, sin2], axis=1).astype(bf16))
    tri = np.triu(np.ones((P, P), dtype=f32)).astype(bf16)  # tri[k,q]=k<=q
    onesd = np.ones((P, P), dtype=bf16)

    in_maps = []
    for core in range(8):
        b, g = divmod(core, KVH)
        hs = g * NH * HD            # first q row for this group
        # the device computes rsq = (sum(q^2) + HD*eps')^(-1/2), i.e. the
        # 1/sqrt(HD) and 1/64 factors come out of the pow; qg supplies
        # gain * QSCALE for q and sqrt(HD) for k.
        qg = np.concatenate([
            q_gain[g * NH:(g + 1) * NH].astype(f32) * QSCALE,
            np.array([HD ** 0.5], dtype=f32)])
        wq_hi, wq_lo = hi_lo_planes(
            (WSCALE * Wq[hs:hs + NH * HD].T).astype(f32))
        wkv = np.concatenate(
            [Wk[g * HD:(g + 1) * HD], Wv[g * HD:(g + 1) * HD]],
            axis=0).T.astype(f32) * WSCALE
        wkv_hi, wkv_lo = hi_lo_planes(wkv)
        wp_hi, wp_lo = hi_lo_planes(
            (WSCALE * Wproj.T[hs:hs + NH * HD]).astype(f32))
        in_maps.append({
            "xp": hilo_planes2(x[b].T.astype(f32)),
            "wq8": wq_hi, "wql": wq_lo,
            "wkv8": wkv_hi, "wkvl": wkv_lo,
            "wph": wp_hi, "wpl": wp_lo,
            "cs2": cs2,
            "qg5": np.ascontiguousarray(
                np.broadcast_to(qg, (P, NH + 1)), dtype=f32),
            "tri": tri, "onesd": onesd,
        })
    return in_maps


def kernel(x, Wq, Wk, Wv, Wproj, q_gain):
    from concourse.bass_utils import run_bass_kernel_spmd

    x = np.asarray(x, dtype=np.float32)
    Wq = np.asarray(Wq, dtype=np.float32)
    Wk = np.asarray(Wk, dtype=np.float32)
    Wv = np.asarray(Wv, dtype=np.float32)
    Wproj = np.asarray(Wproj, dtype=np.float32)
    q_gain = np.asarray(q_gain, dtype=np.float32)

    if "nc" not in _CACHE:
        _CACHE["nc"] = _build_nc()
    nc = _CACHE["nc"]

    in_maps = _host_inputs(x, Wq, Wk, Wv, Wproj, q_gain)
    res = run_bass_kernel_spmd(nc, in_maps, core_ids=list(range(8)))

    out = np.zeros((B, S, D), dtype=np.float32)
    for core in range(8):
        b = core // KVH
        out[b] += res.results[core]["outT"].T.astype(np.float32) * OUT_DESCALE
    return out
